# revision 8
# baseline (speedup 1.0000x reference)
"""Trainium2 Bass kernel for nn_CapsuleNetwork (capsule routing, bilinear_type=2).

Precision note: the routing logits |cw| reach ~230 with top-2 gaps as small
as 0.1, so the softmax acts as a near-argmax: any 16-bit rounding of the
bilinear (hat) flips winners and produces O(1) output errors.  Everything is
therefore computed in fp32.

Strategy (data parallel over batch for compute; w sharded over s for I/O):
  - Host ships raw contiguous slices only (no host transposes):
      x   = item_eb[b0:b1]           [BC, S, H] f32   (batch shard)
      wsh = w[0][s0:s1]              [SG, IH, H] f32  (seq shard, 1/8 of w)
      msk = mask[b0:b1] as f32       [BC, S]
    Total bytes over the host link: 105MB x + 52MB w + 1MB mask.
  - On device, each core PE-transposes its w shard into per-(s,i)
    contiguous [h', j] tiles, then FOUR per-capsule DRAM AllGathers
    rebuild the full transposed w (phase i only waits on gather i, so
    the gathers pipeline behind the compute).  The gather window is
    used to load and PE-transpose x into a resident SBUF tile (xts).
  - Per capsule i (4 sequential phases): PE computes
    hat_i[s] = xts_s^T @ wT[s,i] in fp32 PSUM; routing (3 iterations of
    masked softmax / weighted sum / squash / delta) is split across
    engines: DVE runs fused scalar_tensor_tensor chains, the Activation
    engine produces per-s scaled copies (Copy with scale ptr) that the
    Pool engine accumulates (tensor_add), and the Pool engine also runs
    broadcast-AP tensor_mul + tensor_reduce chunks for part of the
    delta.  (Pool cannot touch PSUM and cannot run TensorScalarPtr, so
    all routing state lives in SBUF and Pool only gets tensor_tensor /
    tensor_reduce work.)

If the single 8-core dispatch fails (collectives need all 8 cores), fall
back to a no-collective variant (w replicated) run as two 4-core groups.
"""

import os
import sys

for _p in ("/opt/trn_rl_repo", "/root/.axon_site/_ro/trn_rl_repo"):
    if os.path.isdir(_p) and _p not in sys.path:
        sys.path.insert(0, _p)

from contextlib import ExitStack

import numpy as np

import concourse.bass as bass
import concourse.masks as masks
import concourse.mybir as mybir
import concourse.tile as tile
from concourse.bass_utils import run_bass_kernel_spmd

B, S, I, H = 1024, 200, 4, 128
IH = I * H
NCORES = 8
BC = B // NCORES  # samples per core
SG = S // NCORES  # seq positions per core (w shard)
EPS = 1e-9
CH = 2  # s-chunk for delta (fallback program only)
SCH = 4  # s-chunk for wt tile DMA in the phase loop
XCH = 8  # s-chunk for the x load in prep
F32 = mybir.dt.float32
ALU = mybir.AluOpType
ACTF = mybir.ActivationFunctionType

# routing engine split (tuned against the CoreSim cost model):
#   accum: DVE does s < ACC_V via fused STT; s >= ACC_V go Act(scale)+Pool(add)
#   delta: DVE does s < DEL_V via STT+accum_out; s >= DEL_V go Pool mul+reduce
ACC_V = 133
DEL_V = 120
DCH = 4  # s-chunk for the Pool delta path; (S - DEL_V) % DCH == 0

_cache = {}


def _legalize_waits(nc):
    """neuronxcc walrus codegen supports one sync-wait slot per TPB
    instruction; Tile emits several.  Split: for each instruction with k>1
    waits, prepend k-1 single-wait NoOps on the same engine (semantically
    identical — the engine blocks on each in turn)."""
    import bass_rust

    uid = [0]
    for func in nc.m.functions:
        for bb in func.blocks:
            insts = bb.instructions  # live view
            out = []
            changed = False
            for ins in insts:
                si = ins.sync_info
                waits = list(si.on_wait) if si is not None else []
                if len(waits) > 1:
                    changed = True
                    for w in waits[:-1]:
                        nop = mybir.InstNoOp(
                            name=f"wsplit-{uid[0]}", ins=[], outs=[]
                        )
                        uid[0] += 1
                        nop.engine = ins.engine
                        nop.sync_info = bass_rust.SyncInfo(
                            on_wait=[w], on_update=[]
                        )
                        out.append(nop)
                    si.on_wait = [waits[-1]]
                    ins.sync_info = si
                out.append(ins)
            if changed:
                insts.clear()
                insts.extend(out)


def _build_program(legalize=True):
    """Primary 8-core program: sharded w + on-device AllGather."""
    nc = bass.Bass(
        target_bir_lowering=False, trn_type="TRN2", num_devices=NCORES
    )
    x_d = nc.declare_dram_parameter("x", [BC, S, H], F32, isOutput=False)
    w_d = nc.declare_dram_parameter("wsh", [SG, IH, H], F32, isOutput=False)
    msk_d = nc.declare_dram_parameter("msk", [BC, S], F32, isOutput=False)
    out_d = nc.declare_dram_parameter("out", [BC, IH], F32, isOutput=True)

    RG = [list(range(NCORES))]

    with ExitStack() as ctx:
        tc = ctx.enter_context(tile.TileContext(nc))

        dpool = ctx.enter_context(
            tc.tile_pool(name="dram", bufs=1, space="DRAM")
        )
        # per-capsule shards/gathers so phase i only waits on gather i
        wbs = [dpool.tile([SG, H, H], F32, name=f"wb{i}") for i in range(I)]
        wgs = [dpool.tile([S, H, H], F32, name=f"wg{i}") for i in range(I)]
        xts_d = dpool.tile([H, S, BC], F32)  # transposed x staging (DRAM)

        small = ctx.enter_context(tc.tile_pool(name="small", bufs=1))
        ident = small.tile([H, H], F32)
        msk_t = small.tile([BC, S], F32)
        junk = small.tile([BC, H], F32)
        junka = small.tile([BC, H], F32)
        epsc = small.tile([BC, 1], F32)
        ones = small.tile([BC, 1], F32)
        stats = small.tile([BC, 12], F32)

        # persistent SBUF routing state (Pool cannot access PSUM)
        rps = ctx.enter_context(tc.tile_pool(name="rps", bufs=1))
        cap = rps.tile([BC, H], F32)
        capg = rps.tile([BC, H], F32)
        cw0 = rps.tile([BC, S], F32)
        cw1 = rps.tile([BC, S], F32)
        sw = rps.tile([BC, S], F32)
        tmpa = [rps.tile([BC, H], F32, name=f"tmpa{i}") for i in range(4)]
        tmpd = [rps.tile([BC, DCH, H], F32, name=f"tmpd{i}") for i in range(2)]

        nc.vector.memset(epsc, EPS)
        nc.vector.memset(ones, 1.0)
        nc.sync.dma_start(out=msk_t, in_=msk_d[:, :])

        # ---- w prep: load full shard, PE-transpose [j,h']->[h',j] ----
        masks.make_identity(nc, ident[:, :])  # gpsimd memset+affine_sel

        with (
            tc.tile_pool(name="wprep", bufs=1) as wpr,
            tc.tile_pool(name="wpsum", bufs=3, space="PSUM") as wps,
        ):
            wn = wpr.tile([H, SG, I, H], F32)  # [j, s, i, h']  50KB/part
            HG = SG // 5
            for g in range(5):  # split the load so transposes start early
                eng = nc.sync if g % 2 == 0 else nc.scalar
                eng.dma_start(
                    out=wn[:, g * HG : (g + 1) * HG],
                    in_=w_d[g * HG : (g + 1) * HG].rearrange(
                        "s (i j) h -> j s i h", i=I
                    ),
                )
            wt4 = wpr.tile([H, SG, I, H], F32)  # [h', s, i, j] 50KB/part
            for k in range(SG):
                for i in range(I):
                    pst = wps.tile([H, H], F32)
                    nc.tensor.transpose(pst, wn[:, k, i, :], ident[:, :])
                    nc.scalar.activation(
                        out=wt4[:, k, i, :], in_=pst, func=ACTF.Copy
                    )
            for i in range(I):
                eng = nc.sync if i % 2 == 0 else nc.scalar
                eng.dma_start(
                    out=wbs[i][:, :, :].rearrange("s h j -> h s j"),
                    in_=wt4[:, :, i, :],
                )

        # ---- per-capsule AllGathers on gpsimd (pipelined: phase i only
        # waits on gather i) ----
        for i in range(I):
            nc.gpsimd.collective_compute(
                "AllGather",
                ALU.bypass,
                replica_groups=RG,
                ins=[wbs[i][:, :, :].opt()],
                outs=[wgs[i][:, :, :].opt()],
            )

        # ---- x prep (fills the gather window): load natural x,
        # PE-transpose, and stage the transposed copy in DRAM (streamed
        # back per phase; SBUF only holds hat resident) ----
        big = ctx.enter_context(tc.tile_pool(name="big", bufs=1))
        hat = big.tile([BC, S, H], F32)  # per-phase hat_i, 100KB/part

        with (
            tc.tile_pool(name="xprep", bufs=2) as xpr,
            tc.tile_pool(name="xpsum", bufs=3, space="PSUM") as xps,
        ):
            for s0 in range(0, S, XCH):
                xn = xpr.tile([BC, XCH, H], F32)
                nc.sync.dma_start(out=xn, in_=x_d[:, s0 : s0 + XCH, :])
                xtb = xpr.tile([H, XCH, BC], F32)
                for k in range(XCH):
                    pst = xps.tile([H, BC], F32)
                    nc.tensor.transpose(pst, xn[:, k, :], ident[:, :])
                    if k % 2 == 0:
                        nc.vector.tensor_copy(out=xtb[:, k, :], in_=pst)
                    else:
                        nc.scalar.activation(
                            out=xtb[:, k, :], in_=pst, func=ACTF.Copy
                        )
                nc.scalar.dma_start(
                    out=xts_d[:, s0 : s0 + XCH, :], in_=xtb
                )

        # ---- 4 phases: bilinear on PE, then routing ----
        negmax1 = stats[:, 0:1]
        sumexp1 = stats[:, 1:2]
        recips1 = stats[:, 2:3]
        nrm1 = stats[:, 3:4]
        sq1 = stats[:, 4:5]
        np11 = stats[:, 5:6]
        den1 = stats[:, 6:7]
        rd1 = stats[:, 7:8]
        factor1 = stats[:, 8:9]

        for phase in range(I):
            jlo = phase * H
            with (
                tc.tile_pool(name=f"wp{phase}", bufs=3) as wp,
                tc.tile_pool(name=f"pm{phase}", bufs=3, space="PSUM") as pm,
            ):
                for s0 in range(0, S, SCH):
                    wtile = wp.tile([H, SCH, H], F32)  # [h', s, j]
                    nc.sync.dma_start(
                        out=wtile,
                        in_=wgs[phase][s0 : s0 + SCH].rearrange(
                            "s h j -> h s j"
                        ),
                    )
                    xtile = wp.tile([H, SCH, BC], F32)  # [h', s, b]
                    nc.scalar.dma_start(
                        out=xtile, in_=xts_d[:, s0 : s0 + SCH, :]
                    )
                    ps4 = pm.tile([BC, SCH, H], F32)
                    for c in range(SCH):
                        nc.tensor.matmul(
                            ps4[:, c, :],
                            lhsT=xtile[:, c, :],
                            rhs=wtile[:, c, :],
                            start=True,
                            stop=True,
                        )
                    nc.scalar.activation(
                        out=hat[:, s0 : s0 + SCH, :], in_=ps4, func=ACTF.Copy
                    )

            # ---------- routing for this capsule ----------
            def softmax(cwx):
                nc.vector.tensor_reduce(
                    out=negmax1,
                    in_=cwx,
                    axis=mybir.AxisListType.X,
                    op=ALU.max,
                    negate=True,
                )
                nc.scalar.activation(
                    out=sw,
                    in_=cwx,
                    func=ACTF.Exp,
                    bias=negmax1,
                    scale=1.0,
                    accum_out=sumexp1,
                )
                nc.vector.reciprocal(out=recips1, in_=sumexp1)
                # sw = (exp * 1/sumexp) * mask   (in place)
                nc.vector.scalar_tensor_tensor(
                    out=sw,
                    in0=sw,
                    scalar=recips1,
                    in1=msk_t,
                    op0=ALU.mult,
                    op1=ALU.mult,
                )

            def squash(it):
                nc.scalar.activation(
                    out=junk, in_=cap, func=ACTF.Square, accum_out=nrm1
                )
                if it == 0:
                    nc.vector.tensor_scalar_mul(nrm1, nrm1, 1.0 / (S * S))
                nc.scalar.activation(
                    out=sq1, in_=nrm1, func=ACTF.Sqrt, bias=epsc, scale=1.0
                )
                nc.vector.tensor_scalar_add(np11, nrm1, 1.0)
                nc.vector.tensor_mul(den1, np11, sq1)
                nc.vector.reciprocal(out=rd1, in_=den1)
                nc.vector.tensor_mul(factor1, nrm1, rd1)
                if it == 0:
                    nc.vector.tensor_scalar_mul(factor1, factor1, 1.0 / S)
                nc.vector.tensor_scalar_mul(cap, cap, factor1)

            for it in range(3):
                if it > 0:
                    softmax(cw0 if it == 1 else cw1)

                def scol(s, _it=it):
                    return (
                        msk_t[:, s : s + 1] if _it == 0 else sw[:, s : s + 1]
                    )

                # cap = sum_s sw_s * hat_s:
                #   Act produces tmpa = sw_s * hat_s, Pool accumulates
                #   into capg for s >= ACC_V; DVE runs a fused STT chain
                #   into cap for s < ACC_V.
                nc.gpsimd.memset(capg, 0.0)
                for k, s in enumerate(range(ACC_V, S)):
                    t = tmpa[k % 4]
                    nc.scalar.activation(
                        out=t, in_=hat[:, s, :], func=ACTF.Copy, scale=scol(s)
                    )
                    nc.gpsimd.tensor_add(capg, capg, t)
                nc.vector.memset(cap, 0.0)
                for s in range(ACC_V):
                    nc.vector.scalar_tensor_tensor(
                        out=cap,
                        in0=hat[:, s, :],
                        scalar=scol(s),
                        in1=cap,
                        op0=ALU.mult,
                        op1=ALU.add,
                    )
                nc.vector.tensor_add(cap, cap, capg)

                squash(it)

                if it < 2:
                    # delta[b,s] = <hat_s, cap>: Pool runs broadcast-AP
                    # muls for s >= DEL_V (the per-chunk h-reduce goes to
                    # Act 3:1 DVE — Pool can only reduce over partitions);
                    # DVE runs fused STT with accum_out for s < DEL_V.
                    cwx = cw0 if it == 0 else cw1
                    capb = cap[:, :].unsqueeze(1).broadcast_to([BC, DCH, H])
                    for k, s0 in enumerate(range(DEL_V, S, DCH)):
                        t = tmpd[k % 2]
                        nc.gpsimd.tensor_mul(
                            t, hat[:, s0 : s0 + DCH, :], capb
                        )
                        if k % 4 == 3:
                            nc.vector.tensor_reduce(
                                out=cwx[:, s0 : s0 + DCH],
                                in_=t,
                                axis=mybir.AxisListType.X,
                                op=ALU.add,
                            )
                        else:
                            for c in range(DCH):
                                nc.scalar.activation(
                                    out=junka,
                                    in_=t[:, c, :],
                                    func=ACTF.Copy,
                                    accum_out=cwx[:, s0 + c : s0 + c + 1],
                                )
                    for s in range(DEL_V):
                        nc.vector.scalar_tensor_tensor(
                            out=junk,
                            in0=hat[:, s, :],
                            scalar=ones,
                            in1=cap,
                            op0=ALU.mult,
                            op1=ALU.mult,
                            accum_out=cwx[:, s : s + 1],
                        )
                    if it == 1:
                        # capsule_weight accumulates: cw1 += cw0
                        nc.vector.tensor_add(cw1, cw1, cw0)

            # write this phase's capsule out (stage via SBUF)
            nc.vector.tensor_copy(out=junk, in_=cap)
            nc.sync.dma_start(out=out_d[:, jlo : jlo + H], in_=junk)

    if legalize:
        _legalize_waits(nc)
    return nc


def _build_program_v0():
    """Fallback (no collectives): w fully replicated, transposed layouts
    prepared on the host.  This is the previously-validated baseline."""
    nc = bass.Bass(target_bir_lowering=False, trn_type="TRN2")
    xt_d = nc.declare_dram_parameter("xt", [S, H, BC], F32, isOutput=False)
    wt_d = nc.declare_dram_parameter("wt", [S, H, IH], F32, isOutput=False)
    msk_d = nc.declare_dram_parameter("msk", [BC, S], F32, isOutput=False)
    out_d = nc.declare_dram_parameter("out", [BC, IH], F32, isOutput=True)

    with ExitStack() as ctx:
        tc = ctx.enter_context(tile.TileContext(nc))

        big = ctx.enter_context(tc.tile_pool(name="big", bufs=1))
        small = ctx.enter_context(tc.tile_pool(name="small", bufs=1))

        xts = big.tile([H, S, BC], F32)
        hat = big.tile([BC, S, H], F32)
        msk_t = small.tile([BC, S], F32)
        cw1 = small.tile([BC, S], F32)
        capx = small.tile([BC, CH, H], F32)
        tmp0 = small.tile([BC, CH, H], F32)
        tmp1 = small.tile([BC, CH, H], F32)
        tmps = [tmp0, tmp1]
        junk = small.tile([BC, H], F32)
        junka = small.tile([BC, H], F32)
        epsc = small.tile([BC, 1], F32)
        stats = small.tile([BC, 12], F32)
        negmax1 = stats[:, 0:1]
        sumexp1 = stats[:, 1:2]
        recips1 = stats[:, 2:3]
        nrm1 = stats[:, 3:4]
        sq1 = stats[:, 4:5]
        np11 = stats[:, 5:6]
        den1 = stats[:, 6:7]
        rd1 = stats[:, 7:8]
        factor1 = stats[:, 8:9]

        nc.vector.memset(epsc, EPS)
        nc.sync.dma_start(out=msk_t, in_=msk_d[:, :])
        nc.sync.dma_start(
            out=xts, in_=xt_d[:, :, :].rearrange("s h b -> h s b")
        )

        for phase in range(I):
            jlo = phase * H
            with (
                tc.tile_pool(name=f"wp{phase}", bufs=3) as wp,
                tc.tile_pool(name=f"pm{phase}", bufs=3, space="PSUM") as pm,
            ):
                for s0 in range(0, S, 4):
                    ps4 = pm.tile([BC, 4, H], F32)
                    for c in range(4):
                        s = s0 + c
                        wt_t = wp.tile([H, H], F32)
                        nc.sync.dma_start(
                            out=wt_t, in_=wt_d[s, :, jlo : jlo + H]
                        )
                        nc.tensor.matmul(
                            ps4[:, c, :],
                            lhsT=xts[:, s, :],
                            rhs=wt_t,
                            start=True,
                            stop=True,
                        )
                    nc.scalar.activation(
                        out=hat[:, s0 : s0 + 4, :], in_=ps4, func=ACTF.Copy
                    )

            with tc.tile_pool(name=f"pr{phase}", bufs=1, space="PSUM") as pr:
                cap = pr.tile([BC, H], F32)
                cw0 = pr.tile([BC, S], F32)
                sw = pr.tile([BC, S], F32)

                def softmax(cwx):
                    nc.vector.tensor_reduce(
                        out=negmax1,
                        in_=cwx,
                        axis=mybir.AxisListType.X,
                        op=ALU.max,
                        negate=True,
                    )
                    nc.scalar.activation(
                        out=sw,
                        in_=cwx,
                        func=ACTF.Exp,
                        bias=negmax1,
                        scale=1.0,
                        accum_out=sumexp1,
                    )
                    nc.vector.reciprocal(out=recips1, in_=sumexp1)
                    nc.vector.scalar_tensor_tensor(
                        out=sw,
                        in0=sw,
                        scalar=recips1,
                        in1=msk_t,
                        op0=ALU.mult,
                        op1=ALU.mult,
                    )

                def squash(it):
                    nc.scalar.activation(
                        out=junk, in_=cap, func=ACTF.Square, accum_out=nrm1
                    )
                    if it == 0:
                        nc.vector.tensor_scalar_mul(nrm1, nrm1, 1.0 / (S * S))
                    nc.scalar.activation(
                        out=sq1, in_=nrm1, func=ACTF.Sqrt, bias=epsc, scale=1.0
                    )
                    nc.vector.tensor_scalar_add(np11, nrm1, 1.0)
                    nc.vector.tensor_mul(den1, np11, sq1)
                    nc.vector.reciprocal(out=rd1, in_=den1)
                    nc.vector.tensor_mul(factor1, nrm1, rd1)
                    if it == 0:
                        nc.vector.tensor_scalar_mul(factor1, factor1, 1.0 / S)
                    nc.vector.tensor_scalar_mul(cap, cap, factor1)

                for it in range(3):
                    if it > 0:
                        softmax(cw0 if it == 1 else cw1)

                    nc.vector.memset(cap, 0.0)
                    for s in range(S):
                        nc.vector.scalar_tensor_tensor(
                            out=cap,
                            in0=hat[:, s, :],
                            scalar=(
                                msk_t[:, s : s + 1]
                                if it == 0
                                else sw[:, s : s + 1]
                            ),
                            in1=cap,
                            op0=ALU.mult,
                            op1=ALU.add,
                        )

                    squash(it)

                    if it < 2:
                        for c in range(CH):
                            nc.vector.tensor_copy(out=capx[:, c, :], in_=cap)
                        cwx = cw0 if it == 0 else cw1
                        for k, s0 in enumerate(range(0, S, CH)):
                            tmp = tmps[k % 2]
                            nc.gpsimd.tensor_mul(
                                tmp,
                                hat[:, s0 : s0 + CH, :],
                                capx,
                            )
                            if k % 3 == 0:
                                for c in range(CH):
                                    nc.scalar.activation(
                                        out=junk,
                                        in_=tmp[:, c, :],
                                        func=ACTF.Copy,
                                        accum_out=cwx[:, s0 + c : s0 + c + 1],
                                    )
                            else:
                                nc.vector.tensor_reduce(
                                    out=cwx[:, s0 : s0 + CH],
                                    in_=tmp,
                                    axis=mybir.AxisListType.X,
                                    op=ALU.add,
                                )
                        if it == 1:
                            nc.vector.tensor_add(cw1, cw1, cw0)

                nc.vector.tensor_copy(out=capx[:, 0, :], in_=cap)
                nc.sync.dma_start(
                    out=out_d[:, jlo : jlo + H], in_=capx[:, 0, :]
                )

    _legalize_waits(nc)
    return nc


def kernel(item_eb: np.ndarray, mask: np.ndarray, w: np.ndarray) -> np.ndarray:
    item_eb = np.ascontiguousarray(np.asarray(item_eb, dtype=np.float32))
    mask_i = np.asarray(mask)
    w = np.ascontiguousarray(np.asarray(w, dtype=np.float32))
    mskf = mask_i.astype(np.float32)  # [B, S]

    if "nc" not in _cache:
        _cache["nc"] = _build_program()
    nc = _cache["nc"]

    w0 = w[0]  # [S, IH, H]
    in_maps = []
    for c in range(NCORES):
        b0, b1 = c * BC, (c + 1) * BC
        in_maps.append(
            {
                "x": item_eb[b0:b1],
                "wsh": w0[c * SG : (c + 1) * SG],
                "msk": mskf[b0:b1],
            }
        )

    outs = [None] * NCORES
    try:
        res = run_bass_kernel_spmd(nc, in_maps, list(range(NCORES)))
        for c in range(NCORES):
            outs[c] = np.asarray(res.results[c]["out"])
    except Exception:
        # The collective program needs all 8 cores in one dispatch; if that
        # fails, fall back to the no-collective baseline in two 4-core
        # groups (w replicated, transposed layouts prepared on host).
        if "nc_v0" not in _cache:
            _cache["nc_v0"] = _build_program_v0()
        nc0 = _cache["nc_v0"]
        xt = np.ascontiguousarray(item_eb.transpose(1, 2, 0))  # [S, H, B]
        wt = np.ascontiguousarray(w0.transpose(0, 2, 1))  # [S, H, IH]
        in_maps0 = []
        for c in range(NCORES):
            b0, b1 = c * BC, (c + 1) * BC
            in_maps0.append(
                {
                    "xt": np.ascontiguousarray(xt[:, :, b0:b1]),
                    "wt": wt,
                    "msk": np.ascontiguousarray(mskf[b0:b1]),
                }
            )
        for grp in ([0, 1, 2, 3], [4, 5, 6, 7]):
            res = run_bass_kernel_spmd(nc0, [in_maps0[c] for c in grp], grp)
            for i, c in enumerate(grp):
                outs[c] = np.asarray(res.results[i]["out"])

    full = np.concatenate(outs, axis=0).astype(np.float32)  # [B, IH]
    return full.reshape(B, I, H)


if __name__ == "__main__":
    rng = np.random.default_rng(0)
    x = rng.standard_normal((B, S, H), dtype=np.float32)
    m = rng.integers(0, 2, size=(B, S)).astype(np.int32)
    ww = rng.standard_normal((1, S, IH, H), dtype=np.float32)
    o = kernel(item_eb=x, mask=m, w=ww)
    print(o.shape, o.dtype, np.abs(o).mean())
